# revision 67
# baseline (speedup 1.0000x reference)
"""DisentangledSeqEncoder Trainium2 kernel, v2.

Pure data-parallel over batch across 8 NeuronCores (B/8 = 64 per core).
239us (baseline) -> 74.0us modeled; device-verified rel err 5.3e-3.

Key ideas:
  - Host sends z in TWO bf16 layouts (natural + d-major transposed), each
    with 8KB-contiguous partition rows so all 16 big DMAs run at the full
    360 B/ns bus rate (one DMA per group per layout, all hoisted so the
    DMA engine streams back-to-back).
  - Every per-(token,batch) reduction is a PE matmul column against the
    transposed z: scores z@Gc, u-col (z+a)@hc, and the four moments
    {Sz, S(z+a), Sz^2, S((z+a)^2)}/64 (via elementwise z*z on ACT/Pool
    and a*z on DVE feeding tiny 1/64-weighted matmuls; alpha-only terms
    come from extra a/64 and a^2/64 matmuls).
  - Centering tricks remove whole op chains exactly: Gc = G - colmean(G)
    kills the mean*colsum(G) score shift; folding C = I - J/64 into the
    q->h matrix (hc = C(I+W)q) kills the mean*sum(h) shift; LN scale
    invariance turns out = LN(bsq + A/S) into LN(S*bsq + A) (no divide).
  - PSUM discipline: start_tensor_calc resets a whole 2KB bank, so every
    accumulating tile is padded to a private bank and only the first
    matmul of each tile generation uses start=True (later first-writes
    auto-zero lazily). All matmul out/stationary APs collapse to a single
    free dim (walrus/ISA requirement).
  - Engine balance (each ~50us): DVE az-product/sk/wt/kmul-even; ACT
    z^2-squares(j0,j1)/exp/rsqrt-chain/scp-copy-odd; Pool z^2(j2,j3)/
    kmul-odd; PE ~5300 small matmuls (14% busy). The k-softmax sum does
    a bf16 2x fold-add (ev[0:8]+ev[8:16]) before the half-width reduce.
    Steady-state DVE efficiency ~91%; runtime = 6.5us DMA/param startup
    + DVE-paced steady state + ~3.2us fixed out-DMA/barrier epilogue.
  - gamma/beta are folded exactly into host-side Gc/g2col; runtime flags
    add ops only for nontrivial beta0/gamma3/beta3/gamma4/beta4.
"""

import numpy as np

EPS = 1e-6
B_FULL, T, D, K = 512, 1024, 64, 16
NCORES = 8
B_CORE = B_FULL // NCORES          # 64
NG = 8                             # batch groups per core
NB = 8                             # batches per group
NI = 8                             # chunks (inner token index)
P = 128                            # partitions

_CACHE = {}


def _setup_act_tables():
    """Reorder act_func_sets so natural_log_exp_and_others is first (avoids
    per-chunk ACT_TABLE_LOAD thrash on real hw)."""
    import os
    import json
    import functools
    import concourse.hw_specs as hw_specs
    import concourse.bacc as bacc

    if getattr(_setup_act_tables, "_done", False):
        return
    from neuronxcc.driver.Job import Job
    from neuronxcc.driver.jobs.support.FindActInfo import findActInfoFile

    src = findActInfoFile(Job.getPackageDir(), "gen3")
    srcdir = os.path.dirname(src)
    info = json.load(open(src))
    sets = info["act_func_sets"]
    sets.sort(key=lambda e: 0 if e["name"] == "natural_log_exp_and_others" else 1)
    dst = "/tmp/act_reordered"
    os.makedirs(dst, exist_ok=True)
    tmp = os.path.join(dst, f"act_info.{os.getpid()}.tmp")
    json.dump(info, open(tmp, "w"))
    os.replace(tmp, os.path.join(dst, "act_info.json"))
    for f in os.listdir(srcdir):
        if f.endswith(".bin") or f.endswith(".json"):
            l = os.path.join(dst, f)
            if f != "act_info.json" and not os.path.exists(l):
                try:
                    os.symlink(os.path.join(srcdir, f), l)
                except FileExistsError:
                    pass
    os.environ["BASS_ACT_ROOT_JSON_PATH"] = os.path.join(dst, "act_info.json")

    orig = hw_specs.get_activation_tables

    @functools.cache
    def patched(arch):
        d = dict(orig(arch))
        items = list(d.items())
        items.sort(key=lambda kv: 0 if kv[0] == "natural_log_exp_and_others"
                   else 1)
        return dict(items)

    hw_specs.get_activation_tables = patched
    bacc.get_activation_tables = patched
    _setup_act_tables._done = True


def _emit(nc, zg_d, zT_d, pbf_d, pf3_d, out_d, flags, bfc, f3c):
    import concourse.tile as tile
    import concourse.bass as bass
    from concourse import mybir

    f32 = mybir.dt.float32
    bf16 = mybir.dt.bfloat16
    OP = mybir.AluOpType
    AF = mybir.ActivationFunctionType
    AX = mybir.AxisListType

    NBF = bfc["_total"]
    NF3 = f3c["_total"]

    with tile.TileContext(nc) as tc:
        with (
            tc.tile_pool(name="singles", bufs=1) as singles,
            tc.tile_pool(name="zn", bufs=8) as znp,
            tc.tile_pool(name="zt", bufs=8) as ztp_pool,
            tc.tile_pool(name="prod", bufs=6) as prod,
            tc.tile_pool(name="sfm", bufs=4) as sfm,
            tc.tile_pool(name="gsb", bufs=3) as gsb,
            tc.tile_pool(name="psS", bufs=3, space="PSUM") as psS,
            tc.tile_pool(name="psC", bufs=2, space="PSUM") as psC,   # scores
            tc.tile_pool(name="psAgg", bufs=3, space="PSUM") as psAgg,
        ):
            # ================= startup =================
            pbf = singles.tile([P, NBF], bf16)
            nc.sync.dma_start(out=pbf, in_=pbf_d[:, :])
            pf3 = singles.tile([P, NF3], f32)
            nc.sync.dma_start(out=pf3, in_=pf3_d[:, :])

            def bfv(name, rows=P):
                off, ncol = bfc[name]
                return pbf[0:rows, off:off + ncol]

            def f3v(name, rows=P):
                off, ncol = f3c[name]
                return pf3[0:rows, off:off + ncol]

            aT2 = bfv("aT2").rearrange("p (i t) -> p i t", i=NI)
            a2T = bfv("a2T").rearrange("p (i t) -> p i t", i=NI)
            rq = bfv("rq")
            raz = bfv("raz")
            ra1 = bfv("ra1")
            raa = bfv("raa")
            RG = bfv("RG")
            WIC = bfv("WIC", rows=D)
            idbf = bfv("idbf", rows=D)
            ones_bf = bfv("ones")

            zlast = f3v("zlast", rows=D)
            ab8rep = f3v("ab8rep", rows=D)
            g2col = f3v("g2col", rows=D)
            ident = f3v("ident", rows=D)
            rep16 = f3v("rep16", rows=NB)
            bsqrep = f3v("bsqrep")

            epsc = singles.tile([P, 1], f32)
            nc.vector.memset(epsc, EPS)

            # ---- q -> hc chain, once for all 64 (g,b) ----
            qin = singles.tile([D, D], f32)
            nc.vector.tensor_add(out=qin, in0=zlast, in1=ab8rep)
            qst = singles.tile([D, 6], f32)
            nc.vector.bn_stats(out=qst, in_=qin)
            qmv = singles.tile([D, 2], f32)
            nc.vector.bn_aggr(out=qmv, in_=qst)
            qlv = singles.tile([D, 1], f32)
            nc.scalar.activation(out=qlv, in_=qmv[:, 1:2], func=AF.Ln,
                                 bias=epsc[0:D], scale=1.0)
            qiv = singles.tile([D, 1], f32)
            nc.scalar.activation(out=qiv, in_=qlv, func=AF.Exp, scale=-0.5)
            q_t = singles.tile([D, D], f32)
            nc.vector.tensor_scalar(out=q_t, in0=qin, scalar1=qmv[:, 0:1],
                                    scalar2=qiv, op0=OP.subtract, op1=OP.mult)
            if flags["use_g3b3"]:
                nc.vector.tensor_mul(out=q_t, in0=q_t, in1=f3v("g3rep", rows=D))
                nc.vector.tensor_add(out=q_t, in0=q_t, in1=f3v("b3rep", rows=D))
            qtpf = psS.tile([P, 512], f32, tag="S2q", name="qtpf")
            qtp = qtpf[0:D, 0:D]
            nc.tensor.transpose(qtp, q_t, ident)
            qts = singles.tile([D, D], bf16)
            nc.scalar.copy(out=qts, in_=qtp)
            h1pf = psS.tile([P, 512], f32, tag="S2q", name="h1pf")
            h1p = h1pf[0:D, 0:D]
            nc.tensor.matmul(h1p, lhsT=WIC, rhs=qts, start=True, stop=True)
            hT8 = singles.tile([D, D], bf16)
            nc.vector.tensor_scalar_mul(out=hT8, in0=h1p, scalar1=g2col)
            # block-diag h columns for the u-col matmuls: [(b2,d), b2', g, j]
            hcpf = psS.tile([P, 512], f32, tag="S2q", name="hcpf")
            hcp = hcpf[:, 0:64].rearrange("p (a g j) -> p a g j", a=2, g=NG)
            nc.tensor.matmul(
                hcp[0:D, 0, :, :].rearrange("p a b -> p (a b)"), lhsT=idbf,
                rhs=hT8[:, 0::2], start=True, stop=True,
                skip_group_check=True)
            nc.tensor.matmul(
                hcp[D:P, 1, :, :].rearrange("p a b -> p (a b)"),
                lhsT=idbf, rhs=hT8[:, 1::2], start=True, stop=True,
                skip_group_check=True)
            nc.vector.memset(hcp[0:D, 1, :, :], 0.0)
            nc.vector.memset(hcp[D:P, 0, :, :], 0.0)
            hcall = singles.tile([P, 2, NG, 4], bf16)
            nc.scalar.copy(out=hcall, in_=hcp)

            # ================= group loop =================
            zTfs, zgfs = [], []
            for g in range(NG):
                zTf = ztp_pool.tile([P, NI * 4 * P], bf16, name=f"zTf{g}",
                                    tag="zTf")
                nc.sync.dma_start(out=zTf, in_=zT_d[g, :, :])
                zgf = znp.tile([P, NB * NI * D], bf16, name=f"zgf{g}",
                               tag="zgf")
                nc.sync.dma_start(out=zgf, in_=zg_d[g, :, :])
                zTfs.append(zTf)
                zgfs.append(zgf)

            def front(st):
                g, qd = st["g"], st["qd"]
                zT = zTfs[g].rearrange("p (j i t) -> p j i t", j=4, i=NI)
                st["zT"] = zT
                st["zg"] = zgfs[g].rearrange("p (b i d) -> p b i d",
                                             b=NB, i=NI)
                if qd == 0:
                    aggcf = psAgg.tile([P, 512], f32, name="aggc")
                    aggc = aggcf[:, 0:NB * K + D + 2]
                    st["aggc"] = aggc
                else:
                    aggc = st["aggc"]
                st["aggp"] = aggc[0:D, 0:NB * K].rearrange(
                    "p (b k) -> p b k", b=NB)
                st["spp"] = aggc[0:NB, NB * K:NB * K + 1]
                hT8g = hT8[:, g * NB:(g + 1) * NB]

                # S2: [P, (ch j), q, b2] ; q = {mz, mza, z2, za2, u}
                S2f = psS.tile([P, 512], f32, tag="S2q", name="S2f")
                S2 = S2f[:, 0:160].rearrange("p (c q b) -> p c q b",
                                             c=16, q=5)
                scp = psC.tile([P, 16, K, 2], f32, name="scp", tag="scp")
                st["S2"], st["scp"] = S2, scp

                for cc in range(2):
                    i0 = 4 * qd + 2 * cc
                    zsl = zT[:, :, i0:i0 + 2, :]
                    asl = aT2[:, i0:i0 + 2, :].unsqueeze(1) \
                        .broadcast_to((P, 4, 2, P))
                    zzp = prod.tile([P, 4, 2, P], bf16, name="zzp",
                                    tag="zzp")
                    azp = prod.tile([P, 4, 2, P], bf16, name="azp",
                                    tag="azp")
                    # z*z : j{0,1} on ACT (Square), j{2,3} on Pool
                    nc.scalar.activation(out=zzp[:, 0:2],
                                         in_=zT[:, 0:2, i0:i0 + 2, :],
                                         func=AF.Square)
                    nc.gpsimd.tensor_mul(out=zzp[:, 2:4],
                                         in0=zT[:, 2:4, i0:i0 + 2, :],
                                         in1=zT[:, 2:4, i0:i0 + 2, :])
                    # a*z on DVE (bf16 2x)
                    nc.vector.tensor_tensor(out=azp, in0=zsl, in1=asl,
                                            op=OP.mult)

                    for c01 in range(2):
                        i = i0 + c01
                        ch = 2 * cc + c01
                        for j in range(4):
                            cj = 4 * ch + j
                            first = cj == 0 and cc == 0
                            last = cj == 15
                            # PSUM start=True resets the whole 2KB bank:
                            # exactly one start per tile generation (the
                            # first matmul); later first-writes auto-zero.
                            nc.tensor.matmul(
                                S2[:, cj, 0:2, :],
                                lhsT=zT[:, j, i, :], rhs=rq,
                                start=first, stop=False,
                                skip_group_check=True)
                            nc.tensor.matmul(
                                scp[:, cj, :, :],
                                lhsT=zT[:, j, i, :], rhs=RG,
                                start=(cj == 0), stop=last,
                                skip_group_check=True)
                            nc.tensor.matmul(
                                S2[:, cj, 4, :],
                                lhsT=zT[:, j, i, :],
                                rhs=hcall[:, :, g, j],
                                start=False, stop=False,
                                skip_group_check=True)
                            # z^2 -> q2,q3 ; a*z -> q3
                            nc.tensor.matmul(
                                S2[:, cj, 2:4, :],
                                lhsT=zzp[:, j, c01, :], rhs=rq,
                                start=False, stop=False,
                                skip_group_check=True)
                            nc.tensor.matmul(
                                S2[:, cj, 3, :],
                                lhsT=azp[:, j, c01, :], rhs=raz,
                                start=False, stop=False,
                                skip_group_check=True)
                            # alpha-only: Sa/64 -> q1, Sa2/64 -> q3
                            nc.tensor.matmul(
                                S2[:, cj, 0:2, :],
                                lhsT=aT2[:, i, :], rhs=ra1,
                                start=False, stop=False,
                                skip_group_check=True)
                            nc.tensor.matmul(
                                S2[:, cj, 2:4, :],
                                lhsT=a2T[:, i, :], rhs=raa,
                                start=False, stop=False,
                                skip_group_check=True)
                            # u-col: + a @ hc
                            nc.tensor.matmul(
                                S2[:, cj, 4, :],
                                lhsT=aT2[0:D, i, :],
                                rhs=hT8g[:, 2 * j:2 * j + 2],
                                start=False, stop=last,
                                skip_group_check=True)

            def back(st):
                g, qd = st["g"], st["qd"]
                S2, scp = st["S2"], st["scp"]
                aggp, spp = st["aggp"], st["spp"]
                zg = st["zg"]
                U = S2[:, :, 4, :]
                # ---- stats: var -> 1/sqrt ----
                sqt = sfm.tile([P, 16, 2, 2], f32, name="sqt", tag="sqt")
                nc.scalar.activation(out=sqt, in_=S2[:, :, 0:2, :],
                                     func=AF.Square)
                vvt = sfm.tile([P, 16, 2, 2], f32, name="vvt", tag="vvt")
                nc.vector.tensor_sub(out=vvt, in0=S2[:, :, 2:4, :],
                                     in1=sqt)
                lnv = sfm.tile([P, 16, 2, 2], f32, name="lnv", tag="lnv")
                nc.scalar.activation(out=lnv, in_=vvt, func=AF.Ln,
                                     bias=epsc, scale=1.0)
                ivq = sfm.tile([P, 16, 2, 2], f32, name="ivq", tag="ivq")
                nc.scalar.activation(out=ivq, in_=lnv, func=AF.Exp,
                                     scale=-0.5)

                # ---- softmax ----
                stile = sfm.tile([P, 16, K + 1, 2], f32, name="stile",
                                 tag="stile")
                inv_z = ivq[:, :, 0, :].unsqueeze(2).broadcast_to(
                    (P, 16, K, 2))
                if qd % 2 == 0 or g == NG - 1:
                    nc.vector.tensor_tensor(out=stile[:, :, 0:K, :],
                                            in0=scp, in1=inv_z,
                                            op=OP.mult)
                else:
                    scpc = sfm.tile([P, 16, K, 2], f32, name="scpc",
                                    tag="scpc")
                    nc.scalar.copy(out=scpc, in_=scp)
                    nc.gpsimd.tensor_tensor(out=stile[:, :, 0:K, :],
                                            in0=scpc, in1=inv_z,
                                            op=OP.mult)
                nc.vector.tensor_tensor(out=stile[:, :, K, :],
                                        in0=U, in1=ivq[:, :, 1, :],
                                        op=OP.mult)
                etile = sfm.tile([P, 16, K + 1, 2], bf16, name="etile",
                                 tag="etile")
                nc.scalar.activation(out=etile, in_=stile, func=AF.Exp)
                ucop = sfm.tile([P, 16, 2], bf16, name="ucop", tag="ucop")
                nc.vector.tensor_copy(out=ucop, in_=etile[:, :, K, :])
                ev = etile[:, :, 0:K, :]
                if flags["use_beta0"]:
                    nc.vector.tensor_tensor(
                        out=ev, in0=ev,
                        in1=f3v("ecbrep").unsqueeze(1).unsqueeze(3)
                        .broadcast_to((P, 16, K, 2)), op=OP.mult)
                with nc.allow_low_precision(reason="bf16 softmax fold"):
                    ev8 = sfm.tile([P, 16, 8, 2], bf16, name="ev8",
                                   tag="ev8")
                    nc.vector.tensor_add(out=ev8, in0=etile[:, :, 0:8, :],
                                         in1=etile[:, :, 8:16, :])
                sk = sfm.tile([P, 16, 2], f32, name="sk", tag="sk")
                nc.vector.reduce_sum(
                    out=sk, in_=ev8.rearrange("p c k b -> p c b k"),
                    axis=AX.X)
                rki = sfm.tile([P, 16, 2], f32, name="rki", tag="rki")
                nc.vector.reciprocal(out=rki, in_=sk)
                rk2 = sfm.tile([P, 16, 2], bf16, name="rk2", tag="rk2")
                nc.vector.tensor_tensor(out=rk2, in0=etile[:, :, K, :],
                                        in1=rki, op=OP.mult)
                wt = sfm.tile([P, 16, K, 2], bf16, name="wt", tag="wt")
                nc.vector.tensor_tensor(
                    out=wt, in0=ev,
                    in1=rk2.unsqueeze(2).broadcast_to((P, 16, K, 2)),
                    op=OP.mult)

                # ---- aggregation + u-sum ----
                for c01 in range(4):
                    i = 4 * qd + c01
                    for b in range(NB):
                        nc.tensor.matmul(
                            aggp[:, b, :], lhsT=zg[:, b, i, :],
                            rhs=wt[:, 4 * c01 + b // 2, :, b % 2],
                            start=(i == 0 and b == 0), stop=(i == NI - 1),
                            skip_group_check=True)
                    nc.tensor.matmul(
                        spp,
                        lhsT=ucop[:, 4 * c01:4 * c01 + 4, :],
                        rhs=ones_bf,
                        start=False, stop=(i == NI - 1),
                        skip_group_check=True)

            def tail(st):
                g = st["g"]
                aggc, spp = st["aggc"], st["spp"]
                atp2 = aggc[:, NB * K + 1:NB * K + 1 + D]
                srp = aggc[:, NB * K + 1 + D:NB * K + 2 + D]
                # LN is scale-invariant: LN(bsq + A/S) = LN(S*bsq + A)
                srec = gsb.tile([NB, 1], f32, name="srec", tag="srec")
                nc.vector.tensor_copy(out=srec, in_=spp)
                nc.tensor.matmul(srp, lhsT=rep16, rhs=srec, start=True,
                                 stop=True)
                srr = gsb.tile([P, 1], f32, name="srr", tag="srr")
                nc.scalar.copy(out=srr, in_=srp)
                ats = gsb.tile([D, NB * K], f32, name="ats", tag="ats")
                nc.scalar.copy(out=ats, in_=aggc[0:D, 0:NB * K])
                nc.tensor.transpose(atp2, ats, ident)
                a8 = gsb.tile([P, D], f32, name="a8", tag="a8")
                nc.vector.scalar_tensor_tensor(
                    out=a8, in0=bsqrep, scalar=srr, in1=atp2,
                    op0=OP.mult, op1=OP.add)
                fst = gsb.tile([P, 6], f32, name="fst", tag="fst")
                nc.vector.bn_stats(out=fst, in_=a8)
                fmv = gsb.tile([P, 2], f32, name="fmv", tag="fmv")
                nc.vector.bn_aggr(out=fmv, in_=fst)
                flv = gsb.tile([P, 1], f32, name="flv", tag="flv")
                nc.scalar.activation(out=flv, in_=fmv[:, 1:2], func=AF.Ln,
                                     bias=epsc, scale=1.0)
                fiv = gsb.tile([P, 1], f32, name="fiv", tag="fiv")
                nc.scalar.activation(out=fiv, in_=flv, func=AF.Exp,
                                     scale=-0.5)
                obuf = gsb.tile([P, D], f32, name="obuf", tag="obuf")
                nc.vector.tensor_scalar(out=obuf, in0=a8,
                                        scalar1=fmv[:, 0:1],
                                        scalar2=fiv, op0=OP.subtract,
                                        op1=OP.mult)
                if flags["use_g4b4"]:
                    nc.vector.tensor_mul(out=obuf, in0=obuf,
                                         in1=f3v("g4rep"))
                    nc.vector.tensor_add(out=obuf, in0=obuf,
                                         in1=f3v("b4rep"))
                nc.sync.dma_start(
                    out=out_d[g * NB:(g + 1) * NB].flatten_outer_dims(),
                    in_=obuf)

            group_state = {}
            for g in range(NG):
                for qd in range(2):
                    st = {"g": g, "qd": qd}
                    if qd == 1:
                        st["aggc"] = group_state[g]["aggc"]
                    front(st)
                    if qd == 0:
                        group_state[g] = st
                    back(st)
                    if qd == 1:
                        tail(group_state.pop(g))

    return nc


def _build(flags):
    import concourse.bacc as bacc
    from concourse import mybir

    _setup_act_tables()
    f32 = mybir.dt.float32
    bf16 = mybir.dt.bfloat16
    bfc, f3c = _param_layouts(flags)
    nc = bacc.Bacc("TRN2", target_bir_lowering=False, debug=False,
                   num_devices=NCORES)
    dp = nc.declare_dram_parameter
    zg_d = dp("zg", [NG, P, NB * NI * D], bf16, isOutput=False)
    zT_d = dp("zT", [NG, P, 4 * NI * P], bf16, isOutput=False)
    pbf_d = dp("pbf", [P, bfc["_total"]], bf16, isOutput=False)
    pf3_d = dp("pf3", [P, f3c["_total"]], f32, isOutput=False)
    out_d = dp("out", [B_CORE, K, D], f32, isOutput=True)
    _emit(nc, zg_d, zT_d, pbf_d, pf3_d, out_d, flags, bfc, f3c)
    nc.finalize()
    return nc


def _param_layouts(flags):
    bfc = {}
    o = 0
    for name, cols in [("aT2", NI * P), ("a2T", NI * P), ("rq", 4),
                       ("raz", 2), ("ra1", 4), ("raa", 4),
                       ("RG", 2 * K), ("idbf", D),
                       ("WIC", D), ("ones", 1)]:
        bfc[name] = (o, cols)
        o += cols
    bfc["_total"] = o
    f3c = {}
    o = 0
    names = [("zlast", D), ("ab8rep", D), ("g2col", 1), ("ident", D),
             ("rep16", P), ("bsqrep", D)]
    if flags["use_beta0"]:
        names.append(("ecbrep", K))
    if flags["use_g3b3"]:
        names += [("g3rep", D), ("b3rep", D)]
    if flags["use_g4b4"]:
        names += [("g4rep", D), ("b4rep", D)]
    for name, cols in names:
        f3c[name] = (o, cols)
        o += cols
    f3c["_total"] = o
    return bfc, f3c


def _ln_np(x, g, b):
    m = x.mean(axis=-1, keepdims=True)
    v = ((x - m) ** 2).mean(axis=-1, keepdims=True)
    return (x - m) / np.sqrt(v + EPS) * g + b


def _host_prep(inputs, flags, bfc, f3c):
    """Shared (non-z) parameter buffers."""
    import ml_dtypes
    bf = ml_dtypes.bfloat16

    al = np.asarray(inputs["alphas"], np.float32)        # [T, D]
    proto = np.asarray(inputs["prototypes"], np.float32)
    bbias = np.asarray(inputs["b_bias"], np.float32)
    W = np.asarray(inputs["W"], np.float32)
    gam = np.asarray(inputs["ln_gamma"], np.float32)
    bet = np.asarray(inputs["ln_beta"], np.float32)
    bseq = np.asarray(inputs["beta_seq"], np.float32)

    pn = _ln_np(proto, gam[1], bet[1])                   # [K, D]
    G = (pn * gam[0]).T / 8.0                            # [D, K]
    Gc = G - G.mean(axis=0, keepdims=True)               # center: kills m*cg

    alp = al.reshape(P, NI, D)                           # [tau, i, d]

    pbf = np.zeros((P, bfc["_total"]), np.float32)

    def put(name, rows, arr):
        off, ncol = bfc[name]
        pbf[0:rows, off:off + ncol] = arr.reshape(rows, ncol)

    aT2h = alp.transpose(2, 1, 0)                        # [d, i, tau]
    aT2h = np.concatenate([aT2h, aT2h], axis=0)          # [128, 8, 128]
    put("aT2", P, aT2h)
    rqh = np.zeros((P, 2, 2), np.float32)                # [p, q01, b2]
    for b2 in range(2):
        rqh[b2 * D:(b2 + 1) * D, :, b2] = 1.0 / 64.0
    put("rq", P, rqh)
    razh = np.zeros((P, 2), np.float32)
    for b2 in range(2):
        razh[b2 * D:(b2 + 1) * D, b2] = 2.0 / 64.0
    put("raz", P, razh)
    RGh = np.zeros((P, K, 2), np.float32)                # [p, k, b2]
    for b2 in range(2):
        RGh[b2 * D:(b2 + 1) * D, :, b2] = Gc
    put("RG", P, RGh)
    a2Th = aT2h ** 2                                     # [128, 8, 128]
    put("a2T", P, a2Th)
    ra1h = np.zeros((P, 2, 2), np.float32)               # q0: 0, q1: 1/64
    for b2 in range(2):
        ra1h[b2 * D:(b2 + 1) * D, 1, b2] = 1.0 / 64.0
    put("ra1", P, ra1h)
    raah = np.zeros((P, 2, 2), np.float32)               # q2: 0, q3: 1/64
    for b2 in range(2):
        raah[b2 * D:(b2 + 1) * D, 1, b2] = 1.0 / 64.0
    put("raa", P, raah)
    put("idbf", D, np.eye(D, dtype=np.float32))
    C = np.eye(D, dtype=np.float32) - 1.0 / 64.0         # I - J/64
    # matmul computes lhsT.T @ rhs -> supply M^T so h1 = M @ qT
    WICh = (C @ (np.eye(D, dtype=np.float32) + W)).T     # hc = C(I+W)q
    put("WIC", D, WICh)
    put("ones", P, np.ones((P, 1), np.float32))
    pbf = pbf.astype(bf)

    pf3 = np.zeros((P, f3c["_total"]), np.float32)

    def putf(name, rows, arr):
        off, ncol = f3c[name]
        pf3[0:rows, off:off + ncol] = arr.reshape(rows, ncol)

    putf("ab8rep", D, np.broadcast_to(al[-1] + bbias, (D, D)).copy())
    putf("g2col", D, (gam[2] / 8.0).reshape(D, 1))
    putf("ident", D, np.eye(D, dtype=np.float32))
    rep16h = np.zeros((NB, P), np.float32)
    for b in range(NB):
        rep16h[b, b * K:(b + 1) * K] = 1.0
    putf("rep16", NB, rep16h)
    putf("bsqrep", P, np.broadcast_to(
        bseq[None, :, :], (NB, K, D)).reshape(P, D).copy())
    if flags["use_beta0"]:
        cb = pn @ bet[0]                                 # [K]
        putf("ecbrep", P, np.broadcast_to(np.exp(cb / 1.0)[None, :],
                                          (P, K)).copy())
    if flags["use_g3b3"]:
        putf("g3rep", D, np.broadcast_to(gam[3], (D, D)).copy())
        putf("b3rep", D, np.broadcast_to(bet[3], (D, D)).copy())
    if flags["use_g4b4"]:
        putf("g4rep", P, np.broadcast_to(gam[4], (P, D)).copy())
        putf("b4rep", P, np.broadcast_to(bet[4], (P, D)).copy())
    return pbf, pf3


def kernel(**inputs):
    import ml_dtypes
    from concourse.bass_utils import run_bass_kernel_spmd

    bf = ml_dtypes.bfloat16
    z = np.ascontiguousarray(inputs["z"], dtype=np.float32)
    gam = np.asarray(inputs["ln_gamma"], np.float32)
    bet = np.asarray(inputs["ln_beta"], np.float32)
    flags = {
        "use_beta0": bool(np.abs(bet[0]).max() > 0),
        "use_g3b3": bool(np.abs(gam[3] - 1).max() > 0
                         or np.abs(bet[3]).max() > 0),
        "use_g4b4": bool(np.abs(gam[4] - 1).max() > 0
                         or np.abs(bet[4]).max() > 0),
        "pool_psum": True,
    }
    key = tuple(sorted(flags.items()))
    if key not in _CACHE:
        _CACHE[key] = _build(flags)
    nc = _CACHE[key]

    bfc, f3c = _param_layouts(flags)
    pbf, pf3_base = _host_prep(inputs, flags, bfc, f3c)

    in_maps = []
    for c in range(NCORES):
        zc = z[c * B_CORE:(c + 1) * B_CORE]              # [64, 1024, 64]
        zc5 = zc.reshape(NG, NB, P, NI, D)
        zg_nat = np.ascontiguousarray(
            zc5.transpose(0, 2, 1, 3, 4)).reshape(NG, P, NB * NI * D)
        zc6 = zc.reshape(NG, 4, 2, P, NI, D)             # [g, j, b2, tau, i, d]
        zT = np.ascontiguousarray(
            zc6.transpose(0, 2, 5, 1, 4, 3)).reshape(NG, P, 4 * NI * P)
        pf3 = pf3_base.copy()
        off, ncol = f3c["zlast"]
        pf3[0:D, off:off + ncol] = zc[:, -1, :]
        in_maps.append({
            "zg": zg_nat.astype(bf),
            "zT": zT.astype(bf),
            "pbf": pbf,
            "pf3": pf3,
        })
    res = run_bass_kernel_spmd(nc, in_maps, core_ids=list(range(NCORES)))
    out = np.concatenate([r["out"] for r in res.results], axis=0)
    return out


# revision 68
# speedup vs baseline: 1.0014x; 1.0014x over previous
"""DisentangledSeqEncoder Trainium2 kernel, v2.

Pure data-parallel over batch across 8 NeuronCores (B/8 = 64 per core).
239us (baseline) -> 74.0us modeled; device-verified rel err 5.3e-3.

Key ideas:
  - Host sends z in TWO bf16 layouts (natural + d-major transposed), each
    with 8KB-contiguous partition rows so all 16 big DMAs run at the full
    360 B/ns bus rate (one DMA per group per layout, all hoisted so the
    DMA engine streams back-to-back).
  - Every per-(token,batch) reduction is a PE matmul column against the
    transposed z: scores z@Gc, u-col (z+a)@hc, and the four moments
    {Sz, S(z+a), Sz^2, S((z+a)^2)}/64 (via elementwise z*z on ACT/Pool
    and a*z on DVE feeding tiny 1/64-weighted matmuls; alpha-only terms
    come from extra a/64 and a^2/64 matmuls).
  - Centering tricks remove whole op chains exactly: Gc = G - colmean(G)
    kills the mean*colsum(G) score shift; folding C = I - J/64 into the
    q->h matrix (hc = C(I+W)q) kills the mean*sum(h) shift; LN scale
    invariance turns out = LN(bsq + A/S) into LN(S*bsq + A) (no divide).
  - PSUM discipline: start_tensor_calc resets a whole 2KB bank, so every
    accumulating tile is padded to a private bank and only the first
    matmul of each tile generation uses start=True (later first-writes
    auto-zero lazily). All matmul out/stationary APs collapse to a single
    free dim (walrus/ISA requirement).
  - Engine balance (each ~50us): DVE az-product/sk/wt/kmul-even; ACT
    z^2-squares(j0,j1)/exp/rsqrt-chain/scp-copy-odd; Pool z^2(j2,j3)/
    kmul-odd; PE ~5300 small matmuls (14% busy). The k-softmax sum does
    a bf16 2x fold-add (ev[0:8]+ev[8:16]) before the half-width reduce.
    Steady-state DVE efficiency ~91%; runtime = 6.5us DMA/param startup
    + DVE-paced steady state + ~3.2us fixed out-DMA/barrier epilogue.
  - gamma/beta are folded exactly into host-side Gc/g2col; runtime flags
    add ops only for nontrivial beta0/gamma3/beta3/gamma4/beta4.
"""

import numpy as np

EPS = 1e-6
B_FULL, T, D, K = 512, 1024, 64, 16
NCORES = 8
B_CORE = B_FULL // NCORES          # 64
NG = 8                             # batch groups per core
NB = 8                             # batches per group
NI = 8                             # chunks (inner token index)
P = 128                            # partitions

_CACHE = {}


def _setup_act_tables():
    """Reorder act_func_sets so natural_log_exp_and_others is first (avoids
    per-chunk ACT_TABLE_LOAD thrash on real hw)."""
    import os
    import json
    import functools
    import concourse.hw_specs as hw_specs
    import concourse.bacc as bacc

    if getattr(_setup_act_tables, "_done", False):
        return
    from neuronxcc.driver.Job import Job
    from neuronxcc.driver.jobs.support.FindActInfo import findActInfoFile

    src = findActInfoFile(Job.getPackageDir(), "gen3")
    srcdir = os.path.dirname(src)
    info = json.load(open(src))
    sets = info["act_func_sets"]
    sets.sort(key=lambda e: 0 if e["name"] == "natural_log_exp_and_others" else 1)
    dst = "/tmp/act_reordered"
    os.makedirs(dst, exist_ok=True)
    tmp = os.path.join(dst, f"act_info.{os.getpid()}.tmp")
    json.dump(info, open(tmp, "w"))
    os.replace(tmp, os.path.join(dst, "act_info.json"))
    for f in os.listdir(srcdir):
        if f.endswith(".bin") or f.endswith(".json"):
            l = os.path.join(dst, f)
            if f != "act_info.json" and not os.path.exists(l):
                try:
                    os.symlink(os.path.join(srcdir, f), l)
                except FileExistsError:
                    pass
    os.environ["BASS_ACT_ROOT_JSON_PATH"] = os.path.join(dst, "act_info.json")

    orig = hw_specs.get_activation_tables

    @functools.cache
    def patched(arch):
        d = dict(orig(arch))
        items = list(d.items())
        items.sort(key=lambda kv: 0 if kv[0] == "natural_log_exp_and_others"
                   else 1)
        return dict(items)

    hw_specs.get_activation_tables = patched
    bacc.get_activation_tables = patched
    _setup_act_tables._done = True


def _emit(nc, zg_d, zT_d, pbf_d, pf3_d, out_d, flags, bfc, f3c):
    import concourse.tile as tile
    import concourse.bass as bass
    from concourse import mybir

    f32 = mybir.dt.float32
    bf16 = mybir.dt.bfloat16
    OP = mybir.AluOpType
    AF = mybir.ActivationFunctionType
    AX = mybir.AxisListType

    NBF = bfc["_total"]
    NF3 = f3c["_total"]

    with tile.TileContext(nc) as tc:
        with (
            tc.tile_pool(name="singles", bufs=1) as singles,
            tc.tile_pool(name="zn", bufs=8) as znp,
            tc.tile_pool(name="zt", bufs=8) as ztp_pool,
            tc.tile_pool(name="prod", bufs=6) as prod,
            tc.tile_pool(name="sfm", bufs=4) as sfm,
            tc.tile_pool(name="gsb", bufs=3) as gsb,
            tc.tile_pool(name="psS", bufs=3, space="PSUM") as psS,
            tc.tile_pool(name="psC", bufs=2, space="PSUM") as psC,   # scores
            tc.tile_pool(name="psAgg", bufs=3, space="PSUM") as psAgg,
        ):
            # ================= startup =================
            pbf = singles.tile([P, NBF], bf16)
            nc.sync.dma_start(out=pbf, in_=pbf_d[:, :])
            pf3 = singles.tile([P, NF3], f32)
            nc.sync.dma_start(out=pf3, in_=pf3_d[:, :])

            def bfv(name, rows=P):
                off, ncol = bfc[name]
                return pbf[0:rows, off:off + ncol]

            def f3v(name, rows=P):
                off, ncol = f3c[name]
                return pf3[0:rows, off:off + ncol]

            aT2 = bfv("aT2").rearrange("p (i t) -> p i t", i=NI)
            a2T = bfv("a2T").rearrange("p (i t) -> p i t", i=NI)
            rq = bfv("rq")
            raz = bfv("raz")
            ra1 = bfv("ra1")
            raa = bfv("raa")
            RG = bfv("RG")
            WIC = bfv("WIC", rows=D)
            idbf = bfv("idbf", rows=D)
            ones_bf = bfv("ones")

            zlast = f3v("zlast", rows=D)
            ab8rep = f3v("ab8rep", rows=D)
            g2col = f3v("g2col", rows=D)
            ident = f3v("ident", rows=D)
            rep16 = f3v("rep16", rows=NB)
            bsqrep = f3v("bsqrep")

            epsc = singles.tile([P, 1], f32)
            nc.vector.memset(epsc, EPS)

            # ---- q -> hc chain, once for all 64 (g,b) ----
            qin = singles.tile([D, D], f32)
            nc.vector.tensor_add(out=qin, in0=zlast, in1=ab8rep)
            qst = singles.tile([D, 6], f32)
            nc.vector.bn_stats(out=qst, in_=qin)
            qmv = singles.tile([D, 2], f32)
            nc.vector.bn_aggr(out=qmv, in_=qst)
            qlv = singles.tile([D, 1], f32)
            nc.scalar.activation(out=qlv, in_=qmv[:, 1:2], func=AF.Ln,
                                 bias=epsc[0:D], scale=1.0)
            qiv = singles.tile([D, 1], f32)
            nc.scalar.activation(out=qiv, in_=qlv, func=AF.Exp, scale=-0.5)
            q_t = singles.tile([D, D], f32)
            nc.vector.tensor_scalar(out=q_t, in0=qin, scalar1=qmv[:, 0:1],
                                    scalar2=qiv, op0=OP.subtract, op1=OP.mult)
            if flags["use_g3b3"]:
                nc.vector.tensor_mul(out=q_t, in0=q_t, in1=f3v("g3rep", rows=D))
                nc.vector.tensor_add(out=q_t, in0=q_t, in1=f3v("b3rep", rows=D))
            qtpf = psS.tile([P, 512], f32, tag="S2q", name="qtpf")
            qtp = qtpf[0:D, 0:D]
            nc.tensor.transpose(qtp, q_t, ident)
            qts = singles.tile([D, D], bf16)
            nc.scalar.copy(out=qts, in_=qtp)
            h1pf = psS.tile([P, 512], f32, tag="S2q", name="h1pf")
            h1p = h1pf[0:D, 0:D]
            nc.tensor.matmul(h1p, lhsT=WIC, rhs=qts, start=True, stop=True)
            hT8 = singles.tile([D, D], bf16)
            nc.vector.tensor_scalar_mul(out=hT8, in0=h1p, scalar1=g2col)
            # block-diag h columns for the u-col matmuls: [(b2,d), b2', g, j]
            hcpf = psS.tile([P, 512], f32, tag="S2q", name="hcpf")
            hcp = hcpf[:, 0:64].rearrange("p (a g j) -> p a g j", a=2, g=NG)
            nc.tensor.matmul(
                hcp[0:D, 0, :, :].rearrange("p a b -> p (a b)"), lhsT=idbf,
                rhs=hT8[:, 0::2], start=True, stop=True,
                skip_group_check=True)
            nc.tensor.matmul(
                hcp[D:P, 1, :, :].rearrange("p a b -> p (a b)"),
                lhsT=idbf, rhs=hT8[:, 1::2], start=True, stop=True,
                skip_group_check=True)
            nc.vector.memset(hcp[0:D, 1, :, :], 0.0)
            nc.vector.memset(hcp[D:P, 0, :, :], 0.0)
            hcall = singles.tile([P, 2, NG, 4], bf16)
            nc.scalar.copy(out=hcall, in_=hcp)

            # ================= group loop =================
            zTfs, zgfs = [], []
            for g in range(NG):
                zTf = ztp_pool.tile([P, NI * 4 * P], bf16, name=f"zTf{g}",
                                    tag="zTf")
                nc.sync.dma_start(out=zTf, in_=zT_d[g, :, :])
                zgf = znp.tile([P, NB * NI * D], bf16, name=f"zgf{g}",
                               tag="zgf")
                nc.sync.dma_start(out=zgf, in_=zg_d[g, :, :])
                zTfs.append(zTf)
                zgfs.append(zgf)

            def front(st):
                g, qd = st["g"], st["qd"]
                zT = zTfs[g].rearrange("p (j i t) -> p j i t", j=4, i=NI)
                st["zT"] = zT
                st["zg"] = zgfs[g].rearrange("p (b i d) -> p b i d",
                                             b=NB, i=NI)
                if qd == 0:
                    aggcf = psAgg.tile([P, 512], f32, name="aggc")
                    aggc = aggcf[:, 0:NB * K + D + 2]
                    st["aggc"] = aggc
                else:
                    aggc = st["aggc"]
                st["aggp"] = aggc[0:D, 0:NB * K].rearrange(
                    "p (b k) -> p b k", b=NB)
                st["spp"] = aggc[0:NB, NB * K:NB * K + 1]
                hT8g = hT8[:, g * NB:(g + 1) * NB]

                # S2: [P, (ch j), q, b2] ; q = {mz, mza, z2, za2, u}
                S2f = psS.tile([P, 512], f32, tag="S2q", name="S2f")
                S2 = S2f[:, 0:160].rearrange("p (c q b) -> p c q b",
                                             c=16, q=5)
                scp = psC.tile([P, 16, K, 2], f32, name="scp", tag="scp")
                st["S2"], st["scp"] = S2, scp

                for cc in range(2):
                    i0 = 4 * qd + 2 * cc
                    zsl = zT[:, :, i0:i0 + 2, :]
                    asl = aT2[:, i0:i0 + 2, :].unsqueeze(1) \
                        .broadcast_to((P, 4, 2, P))
                    zzp = prod.tile([P, 4, 2, P], bf16, name="zzp",
                                    tag="zzp")
                    azp = prod.tile([P, 4, 2, P], bf16, name="azp",
                                    tag="azp")
                    # z*z : j{0,1} on ACT (Square), j{2,3} on Pool
                    nc.scalar.activation(out=zzp[:, 0:2],
                                         in_=zT[:, 0:2, i0:i0 + 2, :],
                                         func=AF.Square)
                    nc.gpsimd.tensor_mul(out=zzp[:, 2:4],
                                         in0=zT[:, 2:4, i0:i0 + 2, :],
                                         in1=zT[:, 2:4, i0:i0 + 2, :])
                    # a*z on DVE (bf16 2x)
                    nc.vector.tensor_tensor(out=azp, in0=zsl, in1=asl,
                                            op=OP.mult)

                    for c01 in range(2):
                        i = i0 + c01
                        ch = 2 * cc + c01
                        for j in range(4):
                            cj = 4 * ch + j
                            first = cj == 0 and cc == 0
                            last = cj == 15
                            # PSUM start=True resets the whole 2KB bank:
                            # exactly one start per tile generation (the
                            # first matmul); later first-writes auto-zero.
                            nc.tensor.matmul(
                                S2[:, cj, 0:2, :],
                                lhsT=zT[:, j, i, :], rhs=rq,
                                start=first, stop=False,
                                skip_group_check=True)
                            nc.tensor.matmul(
                                scp[:, cj, :, :],
                                lhsT=zT[:, j, i, :], rhs=RG,
                                start=(cj == 0), stop=last,
                                skip_group_check=True)
                            nc.tensor.matmul(
                                S2[:, cj, 4, :],
                                lhsT=zT[:, j, i, :],
                                rhs=hcall[:, :, g, j],
                                start=False, stop=False,
                                skip_group_check=True)
                            # z^2 -> q2,q3 ; a*z -> q3
                            nc.tensor.matmul(
                                S2[:, cj, 2:4, :],
                                lhsT=zzp[:, j, c01, :], rhs=rq,
                                start=False, stop=False,
                                skip_group_check=True)
                            nc.tensor.matmul(
                                S2[:, cj, 3, :],
                                lhsT=azp[:, j, c01, :], rhs=raz,
                                start=False, stop=False,
                                skip_group_check=True)
                            # alpha-only: Sa/64 -> q1, Sa2/64 -> q3
                            nc.tensor.matmul(
                                S2[:, cj, 0:2, :],
                                lhsT=aT2[:, i, :], rhs=ra1,
                                start=False, stop=False,
                                skip_group_check=True)
                            nc.tensor.matmul(
                                S2[:, cj, 2:4, :],
                                lhsT=a2T[:, i, :], rhs=raa,
                                start=False, stop=False,
                                skip_group_check=True)
                            # u-col: + a @ hc
                            nc.tensor.matmul(
                                S2[:, cj, 4, :],
                                lhsT=aT2[0:D, i, :],
                                rhs=hT8g[:, 2 * j:2 * j + 2],
                                start=False, stop=last,
                                skip_group_check=True)

            def back(st):
                g, qd = st["g"], st["qd"]
                S2, scp = st["S2"], st["scp"]
                aggp, spp = st["aggp"], st["spp"]
                zg = st["zg"]
                U = S2[:, :, 4, :]
                # ---- stats: var -> 1/sqrt ----
                sqt = sfm.tile([P, 16, 2, 2], f32, name="sqt", tag="sqt")
                nc.scalar.activation(out=sqt, in_=S2[:, :, 0:2, :],
                                     func=AF.Square)
                vvt = sfm.tile([P, 16, 2, 2], f32, name="vvt", tag="vvt")
                nc.vector.tensor_sub(out=vvt, in0=S2[:, :, 2:4, :],
                                     in1=sqt)
                lnv = sfm.tile([P, 16, 2, 2], f32, name="lnv", tag="lnv")
                nc.scalar.activation(out=lnv, in_=vvt, func=AF.Ln,
                                     bias=epsc, scale=1.0)
                ivq = sfm.tile([P, 16, 2, 2], f32, name="ivq", tag="ivq")
                nc.scalar.activation(out=ivq, in_=lnv, func=AF.Exp,
                                     scale=-0.5)

                # ---- softmax (k-major stile/etile: u-row contiguous) ----
                stile = sfm.tile([P, K + 1, 16, 2], f32, name="stile",
                                 tag="stile")
                kview = stile[:, 0:K, :, :].rearrange("p k c b -> p c k b")
                inv_z = ivq[:, :, 0, :].unsqueeze(2).broadcast_to(
                    (P, 16, K, 2))
                if qd % 2 == 0 or g == NG - 1:
                    nc.vector.tensor_tensor(out=kview,
                                            in0=scp, in1=inv_z,
                                            op=OP.mult)
                else:
                    scpc = sfm.tile([P, 16, K, 2], f32, name="scpc",
                                    tag="scpc")
                    nc.scalar.copy(out=scpc, in_=scp)
                    nc.gpsimd.tensor_tensor(out=kview,
                                            in0=scpc, in1=inv_z,
                                            op=OP.mult)
                nc.vector.tensor_tensor(out=stile[:, K, :, :],
                                        in0=U, in1=ivq[:, :, 1, :],
                                        op=OP.mult)
                etile = sfm.tile([P, K + 1, 16, 2], bf16, name="etile",
                                 tag="etile")
                nc.scalar.activation(out=etile, in_=stile, func=AF.Exp)
                ev = etile[:, 0:K, :, :].rearrange("p k c b -> p c k b")
                if flags["use_beta0"]:
                    nc.vector.tensor_tensor(
                        out=ev, in0=ev,
                        in1=f3v("ecbrep").unsqueeze(1).unsqueeze(3)
                        .broadcast_to((P, 16, K, 2)), op=OP.mult)
                with nc.allow_low_precision(reason="bf16 softmax fold"):
                    ev8 = sfm.tile([P, 8, 16, 2], bf16, name="ev8",
                                   tag="ev8")
                    nc.vector.tensor_add(out=ev8, in0=etile[:, 0:8, :, :],
                                         in1=etile[:, 8:16, :, :])
                sk = sfm.tile([P, 16, 2], f32, name="sk", tag="sk")
                nc.vector.reduce_sum(
                    out=sk, in_=ev8.rearrange("p k c b -> p c b k"),
                    axis=AX.X)
                rki = sfm.tile([P, 16, 2], f32, name="rki", tag="rki")
                nc.vector.reciprocal(out=rki, in_=sk)
                rk2 = sfm.tile([P, 16, 2], bf16, name="rk2", tag="rk2")
                nc.vector.tensor_tensor(out=rk2, in0=etile[:, K, :, :],
                                        in1=rki, op=OP.mult)
                wt = sfm.tile([P, K, 16, 2], bf16, name="wt", tag="wt")
                nc.vector.tensor_tensor(
                    out=wt.rearrange("p k c b -> p c k b"), in0=ev,
                    in1=rk2.unsqueeze(2).broadcast_to((P, 16, K, 2)),
                    op=OP.mult)

                # ---- aggregation + u-sum ----
                for c01 in range(4):
                    i = 4 * qd + c01
                    for b in range(NB):
                        nc.tensor.matmul(
                            aggp[:, b, :], lhsT=zg[:, b, i, :],
                            rhs=wt[:, :, 4 * c01 + b // 2, b % 2],
                            start=(i == 0 and b == 0), stop=(i == NI - 1),
                            skip_group_check=True)
                    nc.tensor.matmul(
                        spp,
                        lhsT=etile[:, K, 4 * c01:4 * c01 + 4, :],
                        rhs=ones_bf,
                        start=False, stop=(i == NI - 1),
                        skip_group_check=True)

            def tail(st):
                g = st["g"]
                aggc, spp = st["aggc"], st["spp"]
                atp2 = aggc[:, NB * K + 1:NB * K + 1 + D]
                srp = aggc[:, NB * K + 1 + D:NB * K + 2 + D]
                # LN is scale-invariant: LN(bsq + A/S) = LN(S*bsq + A)
                srec = gsb.tile([NB, 1], f32, name="srec", tag="srec")
                nc.vector.tensor_copy(out=srec, in_=spp)
                nc.tensor.matmul(srp, lhsT=rep16, rhs=srec, start=True,
                                 stop=True)
                srr = gsb.tile([P, 1], f32, name="srr", tag="srr")
                nc.scalar.copy(out=srr, in_=srp)
                ats = gsb.tile([D, NB * K], f32, name="ats", tag="ats")
                nc.scalar.copy(out=ats, in_=aggc[0:D, 0:NB * K])
                nc.tensor.transpose(atp2, ats, ident)
                a8 = gsb.tile([P, D], f32, name="a8", tag="a8")
                nc.vector.scalar_tensor_tensor(
                    out=a8, in0=bsqrep, scalar=srr, in1=atp2,
                    op0=OP.mult, op1=OP.add)
                fst = gsb.tile([P, 6], f32, name="fst", tag="fst")
                nc.vector.bn_stats(out=fst, in_=a8)
                fmv = gsb.tile([P, 2], f32, name="fmv", tag="fmv")
                nc.vector.bn_aggr(out=fmv, in_=fst)
                flv = gsb.tile([P, 1], f32, name="flv", tag="flv")
                nc.scalar.activation(out=flv, in_=fmv[:, 1:2], func=AF.Ln,
                                     bias=epsc, scale=1.0)
                fiv = gsb.tile([P, 1], f32, name="fiv", tag="fiv")
                nc.scalar.activation(out=fiv, in_=flv, func=AF.Exp,
                                     scale=-0.5)
                obuf = gsb.tile([P, D], f32, name="obuf", tag="obuf")
                nc.vector.tensor_scalar(out=obuf, in0=a8,
                                        scalar1=fmv[:, 0:1],
                                        scalar2=fiv, op0=OP.subtract,
                                        op1=OP.mult)
                if flags["use_g4b4"]:
                    nc.vector.tensor_mul(out=obuf, in0=obuf,
                                         in1=f3v("g4rep"))
                    nc.vector.tensor_add(out=obuf, in0=obuf,
                                         in1=f3v("b4rep"))
                nc.sync.dma_start(
                    out=out_d[g * NB:(g + 1) * NB].flatten_outer_dims(),
                    in_=obuf)

            group_state = {}
            for g in range(NG):
                for qd in range(2):
                    st = {"g": g, "qd": qd}
                    if qd == 1:
                        st["aggc"] = group_state[g]["aggc"]
                    front(st)
                    if qd == 0:
                        group_state[g] = st
                    back(st)
                    if qd == 1:
                        tail(group_state.pop(g))

    return nc


def _build(flags):
    import concourse.bacc as bacc
    from concourse import mybir

    _setup_act_tables()
    f32 = mybir.dt.float32
    bf16 = mybir.dt.bfloat16
    bfc, f3c = _param_layouts(flags)
    nc = bacc.Bacc("TRN2", target_bir_lowering=False, debug=False,
                   num_devices=NCORES)
    dp = nc.declare_dram_parameter
    zg_d = dp("zg", [NG, P, NB * NI * D], bf16, isOutput=False)
    zT_d = dp("zT", [NG, P, 4 * NI * P], bf16, isOutput=False)
    pbf_d = dp("pbf", [P, bfc["_total"]], bf16, isOutput=False)
    pf3_d = dp("pf3", [P, f3c["_total"]], f32, isOutput=False)
    out_d = dp("out", [B_CORE, K, D], f32, isOutput=True)
    _emit(nc, zg_d, zT_d, pbf_d, pf3_d, out_d, flags, bfc, f3c)
    nc.finalize()
    return nc


def _param_layouts(flags):
    bfc = {}
    o = 0
    for name, cols in [("aT2", NI * P), ("a2T", NI * P), ("rq", 4),
                       ("raz", 2), ("ra1", 4), ("raa", 4),
                       ("RG", 2 * K), ("idbf", D),
                       ("WIC", D), ("ones", 1)]:
        bfc[name] = (o, cols)
        o += cols
    bfc["_total"] = o
    f3c = {}
    o = 0
    names = [("zlast", D), ("ab8rep", D), ("g2col", 1), ("ident", D),
             ("rep16", P), ("bsqrep", D)]
    if flags["use_beta0"]:
        names.append(("ecbrep", K))
    if flags["use_g3b3"]:
        names += [("g3rep", D), ("b3rep", D)]
    if flags["use_g4b4"]:
        names += [("g4rep", D), ("b4rep", D)]
    for name, cols in names:
        f3c[name] = (o, cols)
        o += cols
    f3c["_total"] = o
    return bfc, f3c


def _ln_np(x, g, b):
    m = x.mean(axis=-1, keepdims=True)
    v = ((x - m) ** 2).mean(axis=-1, keepdims=True)
    return (x - m) / np.sqrt(v + EPS) * g + b


def _host_prep(inputs, flags, bfc, f3c):
    """Shared (non-z) parameter buffers."""
    import ml_dtypes
    bf = ml_dtypes.bfloat16

    al = np.asarray(inputs["alphas"], np.float32)        # [T, D]
    proto = np.asarray(inputs["prototypes"], np.float32)
    bbias = np.asarray(inputs["b_bias"], np.float32)
    W = np.asarray(inputs["W"], np.float32)
    gam = np.asarray(inputs["ln_gamma"], np.float32)
    bet = np.asarray(inputs["ln_beta"], np.float32)
    bseq = np.asarray(inputs["beta_seq"], np.float32)

    pn = _ln_np(proto, gam[1], bet[1])                   # [K, D]
    G = (pn * gam[0]).T / 8.0                            # [D, K]
    Gc = G - G.mean(axis=0, keepdims=True)               # center: kills m*cg

    alp = al.reshape(P, NI, D)                           # [tau, i, d]

    pbf = np.zeros((P, bfc["_total"]), np.float32)

    def put(name, rows, arr):
        off, ncol = bfc[name]
        pbf[0:rows, off:off + ncol] = arr.reshape(rows, ncol)

    aT2h = alp.transpose(2, 1, 0)                        # [d, i, tau]
    aT2h = np.concatenate([aT2h, aT2h], axis=0)          # [128, 8, 128]
    put("aT2", P, aT2h)
    rqh = np.zeros((P, 2, 2), np.float32)                # [p, q01, b2]
    for b2 in range(2):
        rqh[b2 * D:(b2 + 1) * D, :, b2] = 1.0 / 64.0
    put("rq", P, rqh)
    razh = np.zeros((P, 2), np.float32)
    for b2 in range(2):
        razh[b2 * D:(b2 + 1) * D, b2] = 2.0 / 64.0
    put("raz", P, razh)
    RGh = np.zeros((P, K, 2), np.float32)                # [p, k, b2]
    for b2 in range(2):
        RGh[b2 * D:(b2 + 1) * D, :, b2] = Gc
    put("RG", P, RGh)
    a2Th = aT2h ** 2                                     # [128, 8, 128]
    put("a2T", P, a2Th)
    ra1h = np.zeros((P, 2, 2), np.float32)               # q0: 0, q1: 1/64
    for b2 in range(2):
        ra1h[b2 * D:(b2 + 1) * D, 1, b2] = 1.0 / 64.0
    put("ra1", P, ra1h)
    raah = np.zeros((P, 2, 2), np.float32)               # q2: 0, q3: 1/64
    for b2 in range(2):
        raah[b2 * D:(b2 + 1) * D, 1, b2] = 1.0 / 64.0
    put("raa", P, raah)
    put("idbf", D, np.eye(D, dtype=np.float32))
    C = np.eye(D, dtype=np.float32) - 1.0 / 64.0         # I - J/64
    # matmul computes lhsT.T @ rhs -> supply M^T so h1 = M @ qT
    WICh = (C @ (np.eye(D, dtype=np.float32) + W)).T     # hc = C(I+W)q
    put("WIC", D, WICh)
    put("ones", P, np.ones((P, 1), np.float32))
    pbf = pbf.astype(bf)

    pf3 = np.zeros((P, f3c["_total"]), np.float32)

    def putf(name, rows, arr):
        off, ncol = f3c[name]
        pf3[0:rows, off:off + ncol] = arr.reshape(rows, ncol)

    putf("ab8rep", D, np.broadcast_to(al[-1] + bbias, (D, D)).copy())
    putf("g2col", D, (gam[2] / 8.0).reshape(D, 1))
    putf("ident", D, np.eye(D, dtype=np.float32))
    rep16h = np.zeros((NB, P), np.float32)
    for b in range(NB):
        rep16h[b, b * K:(b + 1) * K] = 1.0
    putf("rep16", NB, rep16h)
    putf("bsqrep", P, np.broadcast_to(
        bseq[None, :, :], (NB, K, D)).reshape(P, D).copy())
    if flags["use_beta0"]:
        cb = pn @ bet[0]                                 # [K]
        putf("ecbrep", P, np.broadcast_to(np.exp(cb / 1.0)[None, :],
                                          (P, K)).copy())
    if flags["use_g3b3"]:
        putf("g3rep", D, np.broadcast_to(gam[3], (D, D)).copy())
        putf("b3rep", D, np.broadcast_to(bet[3], (D, D)).copy())
    if flags["use_g4b4"]:
        putf("g4rep", P, np.broadcast_to(gam[4], (P, D)).copy())
        putf("b4rep", P, np.broadcast_to(bet[4], (P, D)).copy())
    return pbf, pf3


def kernel(**inputs):
    import ml_dtypes
    from concourse.bass_utils import run_bass_kernel_spmd

    bf = ml_dtypes.bfloat16
    z = np.ascontiguousarray(inputs["z"], dtype=np.float32)
    gam = np.asarray(inputs["ln_gamma"], np.float32)
    bet = np.asarray(inputs["ln_beta"], np.float32)
    flags = {
        "use_beta0": bool(np.abs(bet[0]).max() > 0),
        "use_g3b3": bool(np.abs(gam[3] - 1).max() > 0
                         or np.abs(bet[3]).max() > 0),
        "use_g4b4": bool(np.abs(gam[4] - 1).max() > 0
                         or np.abs(bet[4]).max() > 0),
        "pool_psum": True,
    }
    key = tuple(sorted(flags.items()))
    if key not in _CACHE:
        _CACHE[key] = _build(flags)
    nc = _CACHE[key]

    bfc, f3c = _param_layouts(flags)
    pbf, pf3_base = _host_prep(inputs, flags, bfc, f3c)

    in_maps = []
    for c in range(NCORES):
        zc = z[c * B_CORE:(c + 1) * B_CORE]              # [64, 1024, 64]
        zc5 = zc.reshape(NG, NB, P, NI, D)
        zg_nat = np.ascontiguousarray(
            zc5.transpose(0, 2, 1, 3, 4)).reshape(NG, P, NB * NI * D)
        zc6 = zc.reshape(NG, 4, 2, P, NI, D)             # [g, j, b2, tau, i, d]
        zT = np.ascontiguousarray(
            zc6.transpose(0, 2, 5, 1, 4, 3)).reshape(NG, P, 4 * NI * P)
        pf3 = pf3_base.copy()
        off, ncol = f3c["zlast"]
        pf3[0:D, off:off + ncol] = zc[:, -1, :]
        in_maps.append({
            "zg": zg_nat.astype(bf),
            "zT": zT.astype(bf),
            "pbf": pbf,
            "pf3": pf3,
        })
    res = run_bass_kernel_spmd(nc, in_maps, core_ids=list(range(NCORES)))
    out = np.concatenate([r["out"] for r in res.results], axis=0)
    return out
